# revision 19
# baseline (speedup 1.0000x reference)
"""Trainium2 Bass kernel for the difflogic LogicLayer problem.

Computation: y = c0 + ca*a + cb*b + cab*a*b where a = x[:, idx_a],
b = x[:, idx_b] and (c0, ca, cb, cab) = softmax(weights) @ GATE_COEFS.

Strategy (8-core SPMD, data-parallel over batch), v8 (out-major):
  - Host marshals x into a transposed bf16 copy per core
    (xt[in, batch], the device-preferred gather layout, like the index
    and coefficient marshalling).
  - Device gathers a/b rows straight from DRAM with non-transposed
    dma_gather (full-rate 4 KiB rows, fused a+b index list per
    256-output chunk) into out-major tiles [128 outs, 2048 batch].
  - Blend out-major: coefficient per partition, so tensor_scalar (DVE
    4x mode) + ACT activation do the affine parts and two 2x
    tensor_tensors finish: 2.7 us DVE per 128x2048 block.
  - PE transposes the bf16 result back to batch-major (psum), ACT
    upconverts to f32, y written with 1 KiB runs.
"""
import numpy as np
import ml_dtypes

import concourse.bacc as bacc
import concourse.mybir as mybir
import concourse.tile as tile
from concourse import masks
from concourse.bass_utils import run_bass_kernel_spmd

# difflogic gate coefficients: rows = gates, cols = (const, a, b, ab)
GATE_COEFS = np.array([
    [0, 0, 0, 0], [0, 0, 0, 1], [0, 1, 0, -1], [0, 1, 0, 0],
    [0, 0, 1, -1], [0, 0, 1, 0], [0, 1, 1, -2], [0, 1, 1, -1],
    [1, -1, -1, 1], [1, -1, -1, 2], [1, 0, -1, 0], [1, 0, -1, 1],
    [1, -1, 0, 0], [1, -1, 0, 1], [1, 0, 0, -1], [1, 0, 0, 0],
], dtype=np.float64)  # [16, 4]

N_CORES = 8
P = 128
BATCH = 16384
IN_DIM = 4096
OUT_DIM = 4096
B = BATCH // N_CORES          # 2048 rows per core
TB = B // P                   # 16 batch blocks
NBLK = OUT_DIM // P           # 32 output blocks
CH = 256                      # outputs per chunk (2 blocks)
NC = OUT_DIM // CH            # 16 chunks
GI = 2 * CH                   # gather idxs per chunk (a then b)
IWC = GI // 16                # wrapped idx cols per chunk

F32 = mybir.dt.float32
BF16 = mybir.dt.bfloat16
I16 = mybir.dt.int16
BF16_NP = ml_dtypes.bfloat16

LAST_EXEC_NS = None
_NC_CACHE = {}


def _build_nc():
    nc = bacc.Bacc("TRN2", target_bir_lowering=False, debug=False,
                   num_devices=N_CORES)
    xt = nc.dram_tensor("xt", [IN_DIM, B], BF16, kind="ExternalInput").ap()
    idx = nc.dram_tensor("idx", [P, NC * IWC], I16,
                         kind="ExternalInput").ap()
    c0d = nc.dram_tensor("c0", [P, NBLK], F32, kind="ExternalInput").ap()
    cad = nc.dram_tensor("ca", [P, NBLK], F32, kind="ExternalInput").ap()
    cbd = nc.dram_tensor("cb", [P, NBLK], F32, kind="ExternalInput").ap()
    cabd = nc.dram_tensor("cab", [P, NBLK], F32, kind="ExternalInput").ap()
    y = nc.dram_tensor("y", [B, OUT_DIM], F32, kind="ExternalOutput").ap()

    mult = mybir.AluOpType.mult
    add = mybir.AluOpType.add
    ident_f = mybir.ActivationFunctionType.Identity

    with tile.TileContext(nc) as tc:
        with tc.tile_pool(name="const", bufs=1) as cpool:
            ident = cpool.tile([P, P], BF16)
            masks.make_identity(nc, ident[:])
            idx_t = cpool.tile([P, NC * IWC], I16, tag="idx")
            nc.sync.dma_start(idx_t[:], idx)
            c0_t = cpool.tile([P, NBLK], F32, tag="c0")
            nc.sync.dma_start(c0_t[:], c0d)
            ca_t = cpool.tile([P, NBLK], F32, tag="ca")
            nc.sync.dma_start(ca_t[:], cad)
            cb_t = cpool.tile([P, NBLK], F32, tag="cb")
            nc.sync.dma_start(cb_t[:], cbd)
            cab_t = cpool.tile([P, NBLK], F32, tag="cab")
            nc.sync.dma_start(cab_t[:], cabd)

            with tc.tile_pool(name="gp", bufs=3) as gp, \
                 tc.tile_pool(name="bp", bufs=3) as bp, \
                 tc.tile_pool(name="ps", bufs=8, space="PSUM") as psp, \
                 tc.tile_pool(name="yp", bufs=3) as yp:
                for c in range(NC):
                    ab = gp.tile([P, 4, B], BF16, tag="ab")
                    nc.gpsimd.dma_gather(
                        ab[:, :, :], xt,
                        idx_t[:, c * IWC:(c + 1) * IWC],
                        GI, GI, B, elem_step=B)
                    yf = yp.tile([P, TB, CH], F32, tag="yf")
                    for u in range(2):       # the 2 output blocks
                        m = 2 * c + u
                        av = ab[:, u, :]
                        bv = ab[:, 2 + u, :]
                        # t1 = cab*b + ca (DVE 4x), t2 = cb*b + c0 (ACT)
                        t1 = bp.tile([P, B], BF16, tag="t1")
                        nc.vector.tensor_scalar(
                            t1[:], bv, cab_t[:, m:m + 1],
                            ca_t[:, m:m + 1], mult, add)
                        t2 = bp.tile([P, B], BF16, tag="t2")
                        nc.scalar.activation(
                            t2[:], bv, ident_f,
                            bias=c0_t[:, m:m + 1], scale=cb_t[:, m:m + 1])
                        # y16 = t1*a + t2 (DVE 2x x2)
                        t3 = bp.tile([P, B], BF16, tag="t3")
                        nc.vector.tensor_mul(t3[:], t1[:], av)
                        y16 = bp.tile([P, B], BF16, tag="y16")
                        nc.vector.tensor_add(y16[:], t3[:], t2[:])
                        # transpose back to batch-major, convert to f32
                        for g in range(TB // 8):
                            ps = psp.tile([P, 8, P], BF16, tag="ps")
                            for q in range(8):
                                tb = g * 8 + q
                                nc.tensor.transpose(
                                    ps[:, q, :],
                                    y16[:, tb * P:(tb + 1) * P],
                                    ident[:])
                            nc.any.tensor_copy(
                                yf[:, g * 8:(g + 1) * 8,
                                   u * P:(u + 1) * P],
                                ps[:, :, :])
                    dst = y[:, c * CH:(c + 1) * CH].rearrange(
                        "(t p) i -> p t i", p=P)
                    nc.sync.dma_start(dst, yf[:, :, :])
    nc.compile()
    return nc


def _wrap_idx(idx_a, idx_b):
    """-> [128, NC*IWC] int16: chunk c's gather k (a for k<CH, b for
    k>=CH) reads wrapped[k % 16, c*IWC + k//16], replicated over the 8
    16-partition groups."""
    ia = np.asarray(idx_a).astype(np.int64)
    ib = np.asarray(idx_b).astype(np.int64)
    seq = np.stack([
        np.concatenate([ia[c * CH:(c + 1) * CH], ib[c * CH:(c + 1) * CH]])
        for c in range(NC)])                       # [NC, GI]
    wr = seq.reshape(NC, IWC, 16).transpose(2, 0, 1)  # [p, c, s]
    wr = wr.reshape(16, NC * IWC).astype(np.int16)
    return np.ascontiguousarray(np.tile(wr, (8, 1)))


def _coef_pt(col):
    """[4096] -> [128, NBLK] f32 with [p, m] = col[m*128 + p]."""
    return np.ascontiguousarray(
        np.asarray(col, dtype=np.float32).reshape(NBLK, P).T)


def kernel(x, weights, idx_a, idx_b, trace=False):
    global LAST_EXEC_NS
    x = np.asarray(x, dtype=np.float32).astype(BF16_NP)
    weights = np.asarray(weights, dtype=np.float64)

    # host: coef table (tiny: [4096, 16] softmax @ [16, 4])
    wmax = weights.max(axis=-1, keepdims=True)
    e = np.exp(weights - wmax)
    wprob = e / e.sum(axis=-1, keepdims=True)
    coef = (wprob @ GATE_COEFS)  # [4096, 4] float64

    idx_w = _wrap_idx(idx_a, idx_b)
    c0 = _coef_pt(coef[:, 0])
    ca = _coef_pt(coef[:, 1])
    cb = _coef_pt(coef[:, 2])
    cab = _coef_pt(coef[:, 3])

    if "nc" not in _NC_CACHE:
        _NC_CACHE["nc"] = _build_nc()
    nc = _NC_CACHE["nc"]

    in_maps = []
    for i in range(N_CORES):
        in_maps.append({
            "xt": np.ascontiguousarray(x[i * B:(i + 1) * B, :].T),
            "idx": idx_w,
            "c0": c0, "ca": ca, "cb": cb, "cab": cab,
        })
    res = run_bass_kernel_spmd(nc, in_maps, core_ids=list(range(N_CORES)),
                               trace=trace)
    LAST_EXEC_NS = res.exec_time_ns
    y = np.concatenate([res.results[i]["y"] for i in range(N_CORES)], axis=0)
    return np.ascontiguousarray(y, dtype=np.float32)


# revision 20
# speedup vs baseline: 1.0630x; 1.0630x over previous
"""Trainium2 Bass kernel for the difflogic LogicLayer problem.

Computation: y = c0 + ca*a + cb*b + cab*a*b where a = x[:, idx_a],
b = x[:, idx_b] and (c0, ca, cb, cab) = softmax(weights) @ GATE_COEFS.

Strategy (8-core SPMD, data-parallel over batch), v8 (out-major):
  - Host marshals x into a transposed bf16 copy per core
    (xt[in, batch], the device-preferred gather layout, like the index
    and coefficient marshalling).
  - Device gathers a/b rows straight from DRAM with non-transposed
    dma_gather (full-rate 4 KiB rows, fused a+b index list per
    256-output chunk) into out-major tiles [128 outs, 2048 batch].
  - Blend out-major: coefficient per partition, so tensor_scalar (DVE
    4x mode) + ACT activation do the affine parts and two 2x
    tensor_tensors finish: 2.7 us DVE per 128x2048 block.
  - PE transposes the bf16 result back to batch-major (psum), ACT
    upconverts to f32, y written with 1 KiB runs.
"""
import numpy as np
import ml_dtypes

import concourse.bacc as bacc
import concourse.mybir as mybir
import concourse.tile as tile
from concourse import masks
from concourse.bass_utils import run_bass_kernel_spmd

# difflogic gate coefficients: rows = gates, cols = (const, a, b, ab)
GATE_COEFS = np.array([
    [0, 0, 0, 0], [0, 0, 0, 1], [0, 1, 0, -1], [0, 1, 0, 0],
    [0, 0, 1, -1], [0, 0, 1, 0], [0, 1, 1, -2], [0, 1, 1, -1],
    [1, -1, -1, 1], [1, -1, -1, 2], [1, 0, -1, 0], [1, 0, -1, 1],
    [1, -1, 0, 0], [1, -1, 0, 1], [1, 0, 0, -1], [1, 0, 0, 0],
], dtype=np.float64)  # [16, 4]

N_CORES = 8
P = 128
BATCH = 16384
IN_DIM = 4096
OUT_DIM = 4096
B = BATCH // N_CORES          # 2048 rows per core
TB = B // P                   # 16 batch blocks
NBLK = OUT_DIM // P           # 32 output blocks
CH = 256                      # outputs per chunk (2 blocks)
NC = OUT_DIM // CH            # 16 chunks
GI = 2 * CH                   # gather idxs per chunk (a then b)
IWC = GI // 16                # wrapped idx cols per chunk

F32 = mybir.dt.float32
BF16 = mybir.dt.bfloat16
I16 = mybir.dt.int16
BF16_NP = ml_dtypes.bfloat16

LAST_EXEC_NS = None
_NC_CACHE = {}


def _build_nc():
    nc = bacc.Bacc("TRN2", target_bir_lowering=False, debug=False,
                   num_devices=N_CORES)
    xt = nc.dram_tensor("xt", [IN_DIM, B], BF16, kind="ExternalInput").ap()
    idx = nc.dram_tensor("idx", [P, NC * IWC], I16,
                         kind="ExternalInput").ap()
    c0d = nc.dram_tensor("c0", [P, NBLK], F32, kind="ExternalInput").ap()
    cad = nc.dram_tensor("ca", [P, NBLK], F32, kind="ExternalInput").ap()
    cbd = nc.dram_tensor("cb", [P, NBLK], F32, kind="ExternalInput").ap()
    cabd = nc.dram_tensor("cab", [P, NBLK], F32, kind="ExternalInput").ap()
    y = nc.dram_tensor("y", [B, OUT_DIM], F32, kind="ExternalOutput").ap()

    mult = mybir.AluOpType.mult
    add = mybir.AluOpType.add
    ident_f = mybir.ActivationFunctionType.Identity

    with tile.TileContext(nc) as tc:
        with tc.tile_pool(name="const", bufs=1) as cpool:
            ident = cpool.tile([P, P], BF16)
            masks.make_identity(nc, ident[:])
            idx_t = cpool.tile([P, NC * IWC], I16, tag="idx")
            nc.sync.dma_start(idx_t[:], idx)
            c0_t = cpool.tile([P, NBLK], F32, tag="c0")
            nc.sync.dma_start(c0_t[:], c0d)
            ca_t = cpool.tile([P, NBLK], F32, tag="ca")
            nc.sync.dma_start(ca_t[:], cad)
            cb_t = cpool.tile([P, NBLK], F32, tag="cb")
            nc.sync.dma_start(cb_t[:], cbd)
            cab_t = cpool.tile([P, NBLK], F32, tag="cab")
            nc.sync.dma_start(cab_t[:], cabd)

            with tc.tile_pool(name="gp", bufs=3) as gp, \
                 tc.tile_pool(name="bp", bufs=3) as bp, \
                 tc.tile_pool(name="ps", bufs=8, space="PSUM") as psp, \
                 tc.tile_pool(name="yp", bufs=3) as yp:
                for c in range(NC):
                    ab = gp.tile([P, 4, B], BF16, tag="ab")
                    nc.gpsimd.dma_gather(
                        ab[:, :, :], xt,
                        idx_t[:, c * IWC:(c + 1) * IWC],
                        GI, GI, B, elem_step=B)
                    yf = yp.tile([P, TB, CH], F32, tag="yf")
                    for u in range(2):       # the 2 output blocks
                        m = 2 * c + u
                        av = ab[:, u, :]
                        bv = ab[:, 2 + u, :]
                        # t1 = cab*b + ca (DVE 4x), t2 = cb*b + c0 (ACT)
                        t1 = bp.tile([P, B], BF16, tag="t1")
                        nc.vector.tensor_scalar(
                            t1[:], bv, cab_t[:, m:m + 1],
                            ca_t[:, m:m + 1], mult, add)
                        t2 = bp.tile([P, B], BF16, tag="t2")
                        nc.vector.tensor_scalar(
                            t2[:], bv, cb_t[:, m:m + 1],
                            c0_t[:, m:m + 1], mult, add)
                        # y16 = t1*a + t2 (DVE 2x x2)
                        t3 = bp.tile([P, B], BF16, tag="t3")
                        nc.vector.tensor_mul(t3[:], t1[:], av)
                        y16 = bp.tile([P, B], BF16, tag="y16")
                        nc.vector.tensor_add(y16[:], t3[:], t2[:])
                        # transpose back to batch-major, convert to f32
                        for g in range(TB // 8):
                            ps = psp.tile([P, 8, P], BF16, tag="ps")
                            for q in range(8):
                                tb = g * 8 + q
                                nc.tensor.transpose(
                                    ps[:, q, :],
                                    y16[:, tb * P:(tb + 1) * P],
                                    ident[:])
                            nc.any.tensor_copy(
                                yf[:, g * 8:(g + 1) * 8,
                                   u * P:(u + 1) * P],
                                ps[:, :, :])
                    dst = y[:, c * CH:(c + 1) * CH].rearrange(
                        "(t p) i -> p t i", p=P)
                    nc.sync.dma_start(dst, yf[:, :, :])
    nc.compile()
    return nc


def _wrap_idx(idx_a, idx_b):
    """-> [128, NC*IWC] int16: chunk c's gather k (a for k<CH, b for
    k>=CH) reads wrapped[k % 16, c*IWC + k//16], replicated over the 8
    16-partition groups."""
    ia = np.asarray(idx_a).astype(np.int64)
    ib = np.asarray(idx_b).astype(np.int64)
    seq = np.stack([
        np.concatenate([ia[c * CH:(c + 1) * CH], ib[c * CH:(c + 1) * CH]])
        for c in range(NC)])                       # [NC, GI]
    wr = seq.reshape(NC, IWC, 16).transpose(2, 0, 1)  # [p, c, s]
    wr = wr.reshape(16, NC * IWC).astype(np.int16)
    return np.ascontiguousarray(np.tile(wr, (8, 1)))


def _coef_pt(col):
    """[4096] -> [128, NBLK] f32 with [p, m] = col[m*128 + p]."""
    return np.ascontiguousarray(
        np.asarray(col, dtype=np.float32).reshape(NBLK, P).T)


def kernel(x, weights, idx_a, idx_b, trace=False):
    global LAST_EXEC_NS
    x = np.asarray(x, dtype=np.float32).astype(BF16_NP)
    weights = np.asarray(weights, dtype=np.float64)

    # host: coef table (tiny: [4096, 16] softmax @ [16, 4])
    wmax = weights.max(axis=-1, keepdims=True)
    e = np.exp(weights - wmax)
    wprob = e / e.sum(axis=-1, keepdims=True)
    coef = (wprob @ GATE_COEFS)  # [4096, 4] float64

    idx_w = _wrap_idx(idx_a, idx_b)
    c0 = _coef_pt(coef[:, 0])
    ca = _coef_pt(coef[:, 1])
    cb = _coef_pt(coef[:, 2])
    cab = _coef_pt(coef[:, 3])

    if "nc" not in _NC_CACHE:
        _NC_CACHE["nc"] = _build_nc()
    nc = _NC_CACHE["nc"]

    in_maps = []
    for i in range(N_CORES):
        in_maps.append({
            "xt": np.ascontiguousarray(x[i * B:(i + 1) * B, :].T),
            "idx": idx_w,
            "c0": c0, "ca": ca, "cb": cb, "cab": cab,
        })
    res = run_bass_kernel_spmd(nc, in_maps, core_ids=list(range(N_CORES)),
                               trace=trace)
    LAST_EXEC_NS = res.exec_time_ns
    y = np.concatenate([res.results[i]["y"] for i in range(N_CORES)], axis=0)
    return np.ascontiguousarray(y, dtype=np.float32)
